# revision 12
# baseline (speedup 1.0000x reference)
"""Causal self-attention (B=4, T=2048, C=1024, H=16, D=64) on 8 trn2 cores.

Sharding: core c -> (batch b = c//2, head-group g = c%2) where a head group is
8 heads = 512 feature columns of each of Q/K/V.  Each core:
  phase 1: QKV projection for its (b, g):  Q^T,K^T [512,2048] fp16, V [2048,512] fp16
  phase 2: causal attention for its 8 heads, scores kept transposed (S^T[k,q])
           so softmax probs feed the AV matmul directly as the moving operand;
           the row-of-ones column appended to V yields the softmax denominator
           in the same matmul.
  phase 3: partial out-projection  Y_partial = O_norm @ W_out[512g:512g+512, :]
Host sums the two partials per batch and adds b_out.

Matmul dtypes: fp32r (full-rate reduced fp32) for the projections, fp16 for the
attention inner matmuls.
"""

from contextlib import ExitStack

import numpy as np

import concourse.bass as bass
import concourse.mybir as mybir
import concourse.tile as tile
from concourse import bacc
from concourse import bass_utils

F32 = mybir.dt.float32
F32R = mybir.dt.float32r
F16 = mybir.dt.float16

B, T, C = 4, 2048, 1024
H, D = 16, 64
G = 2            # head groups (cores per batch)
HPG = 8          # heads per group
CPH = HPG * D    # feature columns per group = 512
N = 512          # matmul moving free dim
NCORES = 8
SCALE = 1.0 / np.sqrt(D)

_CACHE = {}


def _build_program(phases=3):
    key = ("nc", phases)
    if key in _CACHE:
        return _CACHE[key]

    nc = bacc.Bacc("TRN2", target_bir_lowering=False, debug=False, num_devices=NCORES)

    xT = nc.dram_tensor("xT", [C, T], F32R, kind="ExternalInput").ap()
    wq = nc.dram_tensor("wq", [C, CPH], F32R, kind="ExternalInput").ap()
    wk = nc.dram_tensor("wk", [C, CPH], F32R, kind="ExternalInput").ap()
    wv = nc.dram_tensor("wv", [C, CPH], F32R, kind="ExternalInput").ap()
    bq = nc.dram_tensor("bq", [CPH], F32, kind="ExternalInput").ap()
    bk = nc.dram_tensor("bk", [CPH], F32, kind="ExternalInput").ap()
    bv = nc.dram_tensor("bv", [CPH], F32, kind="ExternalInput").ap()
    wo = nc.dram_tensor("wo", [CPH, C], F32R, kind="ExternalInput").ap()
    masks = nc.dram_tensor("masks", [4, 128, N], F16, kind="ExternalInput").ap()
    yp = nc.dram_tensor("yp", [T, C], F32, kind="ExternalOutput").ap()

    with tile.TileContext(nc) as tc, ExitStack() as ctx:
        wpool = ctx.enter_context(tc.tile_pool(name="wpool", bufs=1))
        big = ctx.enter_context(tc.tile_pool(name="big", bufs=1))

        WQ = wpool.tile([128, 8, CPH], F32R)
        WK = wpool.tile([128, 8, CPH], F32R)
        WV = wpool.tile([128, 8, CPH], F32R)
        for cc in range(8):
            nc.sync.dma_start(WQ[:, cc], wq[cc * 128 : (cc + 1) * 128, :])
            nc.sync.dma_start(WK[:, cc], wk[cc * 128 : (cc + 1) * 128, :])
            nc.sync.dma_start(WV[:, cc], wv[cc * 128 : (cc + 1) * 128, :])

        BQ = wpool.tile([128, 4], F32)
        BKs = wpool.tile([128, 4], F32)
        nc.sync.dma_start(BQ[:], bq.rearrange("(o p) -> p o", p=128))
        nc.sync.dma_start(BKs[:], bk.rearrange("(o p) -> p o", p=128))
        # prescale the K bias so S = (Q+bq) . (SCALE*(K+bk))
        nc.vector.tensor_scalar_mul(BKs[:], BKs[:], SCALE)

        MS = wpool.tile([128, 4, N], F16)
        WO = wpool.tile([128, 4, C], F32R)

        QT = big.tile([128, 4, T], F16)   # Q^T (+bias)
        KT = big.tile([128, 4, T], F16)   # SCALE * (K^T + bias)
        VA = big.tile([128, 16, HPG, D + 1], F16)   # V rows + ones column
        ON = big.tile([128, 4, T], F32R)  # normalized O^T (c_in x tokens)
        nc.any.memset(VA[:, :, :, D : D + 1], 1.0)

        _phase1(nc, tc, xT, WQ, WK, WV, BQ, BKs, QT, KT, VA)
        nc.sync.dma_start(MS[:], masks.rearrange("c p q -> p c q"))
        nc.sync.dma_start(WO[:], wo.rearrange("(o p) n -> p o n", p=128))
        if phases >= 2:
            _phase2(nc, tc, QT, KT, VA, ON, MS, WO, yp)
        if phases < 3:
            # debug dump so short builds still produce output
            with tc.tile_pool(name="dbg", bufs=2) as dbg:
                for ic in range(4):
                    t = dbg.tile([128, N], F32, name="dbgt")
                    if phases >= 2:
                        nc.vector.tensor_copy(t[:], ON[:, ic, 0:N])
                    else:
                        nc.vector.tensor_copy(t[:], QT[:, ic, 0:N])
                    nc.sync.dma_start(yp[ic * 128 : ic * 128 + 128, 0:N], t[:])

    nc.compile()
    _CACHE[key] = nc
    return nc


def _phase1(nc, tc, xT, WQ, WK, WV, BQ, BKs, QT, KT, VA):
    with (
        tc.tile_pool(name="xt", bufs=3) as xpool,
        tc.tile_pool(name="ps1", bufs=4, space="PSUM") as ps1,
    ):
        for tb in range(4):
            xt = xpool.tile([128, 8, N], F32R, name="xt")
            for cc in range(8):
                nc.sync.dma_start(
                    xt[:, cc],
                    xT[cc * 128 : (cc + 1) * 128, tb * N : (tb + 1) * N],
                )
            for WT, dst, scl, bias in ((WQ, QT, 1.0, BQ), (WK, KT, SCALE, BKs)):
                for dc in range(4):
                    ps = ps1.tile([128, N], F32, name="ps")
                    for cc in range(8):
                        nc.tensor.matmul(
                            ps[:],
                            WT[:, cc, dc * 128 : (dc + 1) * 128],
                            xt[:, cc],
                            start=(cc == 0),
                            stop=(cc == 7),
                        )
                    nc.vector.scalar_tensor_tensor(
                        out=dst[:, dc, tb * N : (tb + 1) * N],
                        in0=ps[:],
                        scalar=scl,
                        in1=bias[:, dc, None].to_broadcast((128, N)),
                        op0=mybir.AluOpType.mult,
                        op1=mybir.AluOpType.add,
                    )
            for j4 in range(4):
                ps = ps1.tile([128, N], F32, name="ps")
                for cc in range(8):
                    nc.tensor.matmul(
                        ps[:],
                        xt[:, cc, j4 * 128 : (j4 + 1) * 128],
                        WV[:, cc],
                        start=(cc == 0),
                        stop=(cc == 7),
                    )
                jc = tb * 4 + j4
                nc.vector.tensor_copy(
                    VA[:, jc, :, 0:D],
                    ps[:].rearrange("p (h d) -> p h d", h=HPG),
                )


def _phase2(nc, tc, QT, KT, VA, ON, MS, WO, yp):
    with (
        tc.tile_pool(name="et", bufs=6) as epool,
        tc.tile_pool(name="sps", bufs=2, space="PSUM") as sps,
        tc.tile_pool(name="avps", bufs=1, space="PSUM") as avps,
        tc.tile_pool(name="yps", bufs=2, space="PSUM") as yps,
        tc.tile_pool(name="ysb", bufs=4) as ypool,
        tc.tile_pool(name="mpool", bufs=4) as mpool,
    ):
        for qi in range(4):          # query blocks of 512
            nkc = 4 * qi + 4
            for pr in range(4):      # head pairs within the group
                avs = []
                for hi in range(2):
                    av = avps.tile([D + 1, N], F32, name=f"av{hi}")
                    avs.append(av)

                def emit_s(kc):
                    # both heads' score tiles into one 2-bank psum tile so
                    # the exp runs 1024 wide (halves ACT per-op overhead)
                    sp = sps.tile([128, 2 * N], F32, name="sp")
                    for hi in range(2):
                        off = 64 * hi
                        nc.tensor.matmul(
                            sp[:, hi * N : (hi + 1) * N],
                            KT[off : off + 64, pr, kc * 128 : (kc + 1) * 128],
                            QT[off : off + 64, pr, qi * N : (qi + 1) * N],
                            start=True,
                            stop=True,
                        )
                    return sp

                sp_cur = emit_s(0)
                for kc in range(nkc):
                    sp_next = emit_s(kc + 1) if kc + 1 < nkc else None
                    et = epool.tile([128, 2 * N], F16, name="et")
                    nc.scalar.activation(
                        et[:], sp_cur[:], mybir.ActivationFunctionType.Exp
                    )
                    if kc >= 4 * qi:
                        for hi in range(2):
                            nc.vector.tensor_tensor(
                                et[:, hi * N : (hi + 1) * N],
                                et[:, hi * N : (hi + 1) * N],
                                MS[:, kc - 4 * qi, :],
                                mybir.AluOpType.mult,
                            )
                    for hi in range(2):
                        nc.tensor.matmul(
                            avs[hi][:],
                            VA[:, kc, 2 * pr + hi, :],
                            et[:, hi * N : (hi + 1) * N],
                            start=(kc == 0),
                            stop=(kc == nkc - 1),
                        )
                    sp_cur = sp_next
                for hi in range(2):
                    off = 64 * hi
                    dn = mpool.tile([1, N], F32, name="dn")
                    nc.vector.tensor_copy(dn[:], avs[hi][D : D + 1, :])
                    rb = mpool.tile([64, N], F32, name="rb")
                    nc.gpsimd.partition_broadcast(rb[:], dn[:])
                    rc = mpool.tile([64, N], F32, name="rc")
                    nc.vector.reciprocal_approx_fast(rc[:], rb[:])
                    seg = ON[off : off + 64, pr, qi * N : (qi + 1) * N]
                    nc.vector.tensor_tensor(
                        seg, avs[hi][0:64, :], rc[:], mybir.AluOpType.mult
                    )
            # out-projection for this query block (overlaps later qi attention)
            for i4 in range(4):
                ic = 4 * qi + i4
                for ob in range(2):
                    ypt = yps.tile([128, N], F32, name="ypt")
                    for cc4 in range(4):
                        nc.tensor.matmul(
                            ypt[:],
                            ON[:, cc4, ic * 128 : (ic + 1) * 128],
                            WO[:, cc4, ob * N : (ob + 1) * N],
                            start=(cc4 == 0),
                            stop=(cc4 == 3),
                        )
                    ysb = ypool.tile([128, N], F32, name="ysb")
                    nc.vector.tensor_copy(ysb[:], ypt[:])
                    nc.sync.dma_start(
                        yp[ic * 128 : (ic + 1) * 128, ob * N : (ob + 1) * N],
                        ysb[:],
                    )


def _make_masks():
    kp = np.arange(128)[:, None]
    qf = np.arange(N)[None, :]
    return np.stack([(qf >= kp + 128 * c) for c in range(4)]).astype(np.float16)


def _make_in_maps(x, W_qkv, b_qkv, W_out):
    x = np.asarray(x, dtype=np.float32)
    W_qkv = np.asarray(W_qkv, dtype=np.float32)
    b_qkv = np.asarray(b_qkv, dtype=np.float32)
    W_out = np.asarray(W_out, dtype=np.float32)
    masks = _make_masks()
    xT = [np.ascontiguousarray(x[b].T) for b in range(B)]
    in_maps = []
    for c in range(NCORES):
        b, g = c // G, c % G
        lo = CPH * g
        in_maps.append(
            {
                "xT": xT[b],
                "wq": np.ascontiguousarray(W_qkv[:, lo : lo + CPH]),
                "wk": np.ascontiguousarray(W_qkv[:, C + lo : C + lo + CPH]),
                "wv": np.ascontiguousarray(W_qkv[:, 2 * C + lo : 2 * C + lo + CPH]),
                "bq": np.ascontiguousarray(b_qkv[lo : lo + CPH]),
                "bk": np.ascontiguousarray(b_qkv[C + lo : C + lo + CPH]),
                "bv": np.ascontiguousarray(b_qkv[2 * C + lo : 2 * C + lo + CPH]),
                "wo": np.ascontiguousarray(W_out[lo : lo + CPH, :]),
                "masks": masks,
            }
        )
    return in_maps


def _gather(results, b_out, bias_extra):
    bias = np.asarray(b_out, dtype=np.float32) + bias_extra
    out = np.empty((B, T, C), np.float32)
    for b in range(B):
        out[b] = results[G * b]["yp"] + results[G * b + 1]["yp"] + bias[None, :]
    return out


def kernel(x, W_qkv, b_qkv, W_out, b_out, **_):
    nc = _build_program()
    in_maps = _make_in_maps(x, W_qkv, b_qkv, W_out)
    res = bass_utils.run_bass_kernel_spmd(nc, in_maps, core_ids=list(range(NCORES)))
    bias_extra = np.asarray(b_qkv, np.float32)[2 * C :] @ np.asarray(W_out, np.float32)
    return _gather(res.results, b_out, bias_extra)


def kernel_traced(x, W_qkv, b_qkv, W_out, b_out, tmpdir=None, phases=3, trace=True, **_):
    """Like kernel() but returns (out, exec_time_ns); used by test.py."""
    nc = _build_program(phases)
    in_maps = _make_in_maps(x, W_qkv, b_qkv, W_out)
    res = bass_utils.run_bass_kernel_spmd(
        nc, in_maps, core_ids=list(range(NCORES)), trace=trace, tmpdir=tmpdir
    )
    bias_extra = np.asarray(b_qkv, np.float32)[2 * C :] @ np.asarray(W_out, np.float32)
    return _gather(res.results, b_out, bias_extra), res.exec_time_ns


# revision 28
# speedup vs baseline: 1.6121x; 1.6121x over previous
"""Causal self-attention (B=4, T=2048, C=1024, H=16, D=64) on 8 trn2 cores.

Sharding: core c -> (batch b = c//2, head-group g = c%2); a head group is
8 heads = 512 feature columns of each of Q/K/V.  Per core, one fully
software-pipelined program:

  - QKV projection blocks produce Q^T/K^T [64,2048] fp16 per head and
    V [2048,64] fp16 (+ a ones column that makes the AV matmul emit the
    softmax denominator for free).
  - Scores stay transposed (S^T[k,q]) so exp(S^T) feeds the AV matmul as
    the moving operand with no transposes anywhere.
  - The attention stream is ACT(exp)-bound, so the next token-block's
    projection matmuls and the previous query-block's out-projection are
    drip-fed as 2-matmul micro-chunks inside the attention kc-loop to fill
    TensorE slack.

Host pre-arranges inputs partition-major (fp16) and sums the two per-batch
partials, folding b_out + b_v @ W_out (exact: softmax rows sum to 1).

All matmuls run fp16 with fp32 PSUM accumulation.
"""

from collections import deque
from contextlib import ExitStack

import numpy as np

import concourse.bass as bass
import concourse.mybir as mybir
import concourse.tile as tile
from concourse import bacc
from concourse import bass_utils

F32 = mybir.dt.float32
F16 = mybir.dt.float16

B, T, C = 4, 2048, 1024
H, D = 16, 64
G = 2            # head groups (cores per batch)
HPG = 8          # heads per group
CPH = HPG * D    # feature columns per group = 512
N = 512          # matmul moving free dim
NCORES = 8
SCALE = 1.0 / np.sqrt(D)

_CACHE = {}


def _build_program():
    if "nc" in _CACHE:
        return _CACHE["nc"]

    nc = bacc.Bacc("TRN2", target_bir_lowering=False, debug=False, num_devices=NCORES)

    # all inputs pre-arranged host-side: partition-major, fp16
    xTr = nc.dram_tensor("xTr", [128, 8, T], F16, kind="ExternalInput").ap()
    wqr = nc.dram_tensor("wqr", [128, 8, CPH], F16, kind="ExternalInput").ap()
    wkr = nc.dram_tensor("wkr", [128, 8, CPH], F16, kind="ExternalInput").ap()
    wvr = nc.dram_tensor("wvr", [128, 8, CPH], F16, kind="ExternalInput").ap()
    bqr = nc.dram_tensor("bqr", [128, 4], F32, kind="ExternalInput").ap()
    bkr = nc.dram_tensor("bkr", [128, 4], F32, kind="ExternalInput").ap()
    wor = nc.dram_tensor("wor", [128, 4, C], F16, kind="ExternalInput").ap()
    masks = nc.dram_tensor("masks", [128, 4, N], F16, kind="ExternalInput").ap()
    yp = nc.dram_tensor("yp", [T, C], F16, kind="ExternalOutput").ap()

    with tile.TileContext(nc) as tc, ExitStack() as ctx:
        wpool = ctx.enter_context(tc.tile_pool(name="wpool", bufs=1))
        big = ctx.enter_context(tc.tile_pool(name="big", bufs=1))
        epool = ctx.enter_context(tc.tile_pool(name="et", bufs=6))
        mpool = ctx.enter_context(tc.tile_pool(name="mpool", bufs=4))
        blkps = ctx.enter_context(tc.tile_pool(name="blkps", bufs=2, space="PSUM"))
        sps = ctx.enter_context(tc.tile_pool(name="sps", bufs=2, space="PSUM"))
        avps = ctx.enter_context(tc.tile_pool(name="avps", bufs=1, space="PSUM"))

        XT = big.tile([128, 8, T], F16)   # x^T resident (c-chunks x tokens)
        QT = big.tile([128, 4, T], F16)   # Q^T (+bias)
        KT = big.tile([128, 4, T], F16)   # SCALE * (K^T + bias)
        VA = big.tile([128, 16, HPG, D + 1], F16)   # V rows + ones column
        ON = big.tile([128, 4, T], F16)   # normalized O^T (c_in x tokens)

        WQ = wpool.tile([128, 8, CPH], F16)
        WK = wpool.tile([128, 8, CPH], F16)
        WV = wpool.tile([128, 8, CPH], F16)
        BQ = wpool.tile([128, 4], F32)
        BKs = wpool.tile([128, 4], F32)
        MS = wpool.tile([128, 4, N], F16)
        WO = wpool.tile([128, 4, C], F16)

        # input DMAs in first-use order; x/W interleaved in c-chunk pairs so
        # the first projection block can start after ~2MB
        for cc2 in range(4):
            sl = slice(2 * cc2, 2 * cc2 + 2)
            nc.sync.dma_start(XT[:, sl, :], xTr[:, sl, :])
            nc.sync.dma_start(WQ[:, sl, :], wqr[:, sl, :])
            nc.sync.dma_start(WK[:, sl, :], wkr[:, sl, :])
            nc.sync.dma_start(WV[:, sl, :], wvr[:, sl, :])
        nc.sync.dma_start(BQ[:], bqr)
        nc.sync.dma_start(BKs[:], bkr)
        nc.vector.tensor_scalar_mul(BKs[:], BKs[:], SCALE)
        nc.sync.dma_start(MS[:], masks)
        nc.any.memset(VA[:, :, :, D : D + 1], 1.0)
        nc.sync.dma_start(WO[:], wor)

        def qkv_block_gen(tb, which, dc):
            """Generator: one projection block, yielding every 2 matmuls."""
            ps = blkps.tile([128, N], F32, name="blk")
            if which == "v":
                for cc in range(8):
                    nc.tensor.matmul(
                        ps[:],
                        XT[:, cc, tb * N + dc * 128 : tb * N + (dc + 1) * 128],
                        WV[:, cc],
                        start=(cc == 0),
                        stop=(cc == 7),
                    )
                    if cc % 2 == 1:
                        yield
                nc.vector.tensor_copy(
                    VA[:, tb * 4 + dc, :, 0:D],
                    ps[:].rearrange("p (h d) -> p h d", h=HPG),
                )
            else:
                WT, dst, scl, bias = (
                    (WQ, QT, 1.0, BQ) if which == "q" else (WK, KT, SCALE, BKs)
                )
                for cc in range(8):
                    nc.tensor.matmul(
                        ps[:],
                        WT[:, cc, dc * 128 : (dc + 1) * 128],
                        XT[:, cc, tb * N : (tb + 1) * N],
                        start=(cc == 0),
                        stop=(cc == 7),
                    )
                    if cc % 2 == 1:
                        yield
                nc.vector.scalar_tensor_tensor(
                    out=dst[:, dc, tb * N : (tb + 1) * N],
                    in0=ps[:],
                    scalar=scl,
                    in1=bias[:, dc, None].to_broadcast((128, N)),
                    op0=mybir.AluOpType.mult,
                    op1=mybir.AluOpType.add,
                )

        def y_block_gen(ic, ob):
            """Generator: one out-projection block, yielding every 2 matmuls."""
            ypt = blkps.tile([128, N], F32, name="blk")
            for cc4 in range(4):
                nc.tensor.matmul(
                    ypt[:],
                    ON[:, cc4, ic * 128 : (ic + 1) * 128],
                    WO[:, cc4, ob * N : (ob + 1) * N],
                    start=(cc4 == 0),
                    stop=(cc4 == 3),
                )
                if cc4 % 2 == 1:
                    yield
            ysb = mpool.tile([128, N], F16, name="ysb")
            nc.vector.tensor_copy(ysb[:], ypt[:])
            nc.sync.dma_start(
                yp[ic * 128 : (ic + 1) * 128, ob * N : (ob + 1) * N], ysb[:]
            )

        drip = deque()

        def drip_advance(n):
            for _ in range(n):
                while drip:
                    try:
                        next(drip[0])
                        break
                    except StopIteration:
                        drip.popleft()
                else:
                    return

        def drip_drain():
            while drip:
                drip_advance(1)

        def ph1_gens(tb):
            # Q chunks first (needed at the start of query-block tb), then
            # K and V (needed from kc=4*tb onwards)
            return (
                [qkv_block_gen(tb, "q", dc) for dc in range(4)]
                + [qkv_block_gen(tb, "k", dc) for dc in range(4)]
                + [qkv_block_gen(tb, "v", dc) for dc in range(4)]
            )

        # token-block 0 projection up front, ordered so the first head
        # pair's attention dependencies complete as early as possible
        order0 = [("q", 0), ("k", 0), ("v", 0), ("v", 1), ("v", 2), ("v", 3),
                  ("q", 1), ("k", 1), ("q", 2), ("k", 2), ("q", 3), ("k", 3)]
        drip.extend(qkv_block_gen(0, w, dc) for w, dc in order0)
        drip_drain()

        # drip rate per attention iteration, as a fraction (num/den):
        # early query blocks must swallow the next token-block's projection
        # (dependency-forced, PE-bound); the out-projection backlog waits for
        # query-block 3 where the ACT engine has slack
        DRIP_BUDGET = {0: (3, 1), 1: (3, 2), 2: (1, 1), 3: (3, 4)}
        y_backlog = []
        for qi in range(4):
            nkc = 4 * qi + 4
            if qi < 3:
                drip_drain()  # safety: previous token block must be complete
                drip.extend(ph1_gens(qi + 1))
            else:
                drip.extend(y_backlog)
                y_backlog = []
            bnum, bden = DRIP_BUDGET[qi]
            bacc_ctr = 0
            for pr in range(4):
                avs = []
                for hi in range(2):
                    av = avps.tile([D + 1, N], F32, name=f"av{hi}")
                    avs.append(av)

                def emit_s(kc, pr=pr, qi=qi):
                    # both heads' score tiles in one 2-bank psum tile so the
                    # exp runs 1024 wide; the two matmuls run concurrently
                    # (row groups 0-1 / 2-3).  Diagonal chunks only compute
                    # the causally-reachable column range [vq:].
                    vq = max(0, (kc - 4 * qi) * 128)
                    sp = sps.tile([128, 2 * N], F32, name="sp")
                    for hi in range(2):
                        off = 64 * hi
                        nc.tensor.matmul(
                            sp[:, hi * N + vq : (hi + 1) * N],
                            KT[off : off + 64, pr, kc * 128 : (kc + 1) * 128],
                            QT[off : off + 64, pr, qi * N + vq : (qi + 1) * N],
                            start=True,
                            stop=True,
                        )
                    return sp

                sp_cur = emit_s(0)
                for kc in range(nkc):
                    sp_next = emit_s(kc + 1) if kc + 1 < nkc else None
                    bacc_ctr += bnum
                    drip_advance(bacc_ctr // bden)
                    bacc_ctr %= bden
                    et = epool.tile([128, 2 * N], F16, name="et")
                    vq = max(0, (kc - 4 * qi) * 128)
                    if vq == 0:
                        nc.scalar.activation(
                            et[:], sp_cur[:], mybir.ActivationFunctionType.Exp
                        )
                    else:
                        for hi in range(2):
                            nc.scalar.activation(
                                et[:, hi * N + vq : (hi + 1) * N],
                                sp_cur[:, hi * N + vq : (hi + 1) * N],
                                mybir.ActivationFunctionType.Exp,
                            )
                    if kc >= 4 * qi:
                        for hi in range(2):
                            nc.vector.tensor_tensor(
                                et[:, hi * N + vq : (hi + 1) * N],
                                et[:, hi * N + vq : (hi + 1) * N],
                                MS[:, kc - 4 * qi, vq:],
                                mybir.AluOpType.mult,
                            )
                    for hi in range(2):
                        nc.tensor.matmul(
                            avs[hi][:, vq:],
                            VA[:, kc, 2 * pr + hi, :],
                            et[:, hi * N + vq : (hi + 1) * N],
                            start=(kc == 0),
                            stop=(kc == nkc - 1),
                        )
                    sp_cur = sp_next

                for hi in range(2):
                    off = 64 * hi
                    # one copy releases the accumulator bank; the rest of the
                    # normalize chain runs off SBUF, off the critical path
                    oc = mpool.tile([D + 1, N], F32, name="oc")
                    nc.vector.tensor_copy(oc[:], avs[hi][:])
                    dn = mpool.tile([1, N], F32, name="dn")
                    nc.vector.tensor_copy(dn[:], oc[D : D + 1, :])
                    rb = mpool.tile([64, N], F32, name="rb")
                    nc.gpsimd.partition_broadcast(rb[:], dn[:])
                    rc = mpool.tile([64, N], F32, name="rc")
                    nc.vector.reciprocal_approx_fast(rc[:], rb[:])
                    seg = ON[off : off + 64, pr, qi * N : (qi + 1) * N]
                    nc.vector.tensor_tensor(
                        seg, oc[0:64, :], rc[:], mybir.AluOpType.mult
                    )
            # out-projection blocks wait in the backlog for ACT slack
            y_backlog += [
                y_block_gen(4 * qi + i4, ob) for i4 in range(4) for ob in range(2)
            ]
        drip.extend(y_backlog)
        drip_drain()

    nc.compile()
    _CACHE["nc"] = nc
    return nc


def _make_masks():
    kp = np.arange(128)[:, None]
    qf = np.arange(N)[None, :]
    m = np.stack([(qf >= kp + 128 * c) for c in range(4)], axis=1)  # [128,4,N]
    return np.ascontiguousarray(m.astype(np.float16))


def _pm(a, chunks):
    """[chunks*128, F] -> partition-major [128, chunks, F] fp16, contiguous."""
    f = a.shape[-1]
    return np.ascontiguousarray(
        a.reshape(chunks, 128, f).transpose(1, 0, 2).astype(np.float16)
    )


def _make_in_maps(x, W_qkv, b_qkv, W_out):
    x = np.asarray(x, dtype=np.float32)
    W_qkv = np.asarray(W_qkv, dtype=np.float32)
    b_qkv = np.asarray(b_qkv, dtype=np.float32)
    W_out = np.asarray(W_out, dtype=np.float32)
    masks = _make_masks()
    xTr = [_pm(x[b].T, 8) for b in range(B)]
    in_maps = []
    for c in range(NCORES):
        b, g = c // G, c % G
        lo = CPH * g
        bqr = np.ascontiguousarray(
            b_qkv[lo : lo + CPH].reshape(4, 128).T.astype(np.float32)
        )
        bkr = np.ascontiguousarray(
            b_qkv[C + lo : C + lo + CPH].reshape(4, 128).T.astype(np.float32)
        )
        in_maps.append(
            {
                "xTr": xTr[b],
                "wqr": _pm(W_qkv[:, lo : lo + CPH], 8),
                "wkr": _pm(W_qkv[:, C + lo : C + lo + CPH], 8),
                "wvr": _pm(W_qkv[:, 2 * C + lo : 2 * C + lo + CPH], 8),
                "bqr": bqr,
                "bkr": bkr,
                "wor": _pm(W_out[lo : lo + CPH, :], 4),
                "masks": masks,
            }
        )
    return in_maps


def _gather(results, b_out, bias_extra):
    bias = np.asarray(b_out, dtype=np.float32) + bias_extra
    out = np.empty((B, T, C), np.float32)
    for b in range(B):
        out[b] = (
            results[G * b]["yp"].astype(np.float32)
            + results[G * b + 1]["yp"].astype(np.float32)
            + bias[None, :]
        )
    return out


def kernel(x, W_qkv, b_qkv, W_out, b_out, **_):
    nc = _build_program()
    in_maps = _make_in_maps(x, W_qkv, b_qkv, W_out)
    res = bass_utils.run_bass_kernel_spmd(nc, in_maps, core_ids=list(range(NCORES)))
    bias_extra = np.asarray(b_qkv, np.float32)[2 * C :] @ np.asarray(W_out, np.float32)
    return _gather(res.results, b_out, bias_extra)


def kernel_traced(x, W_qkv, b_qkv, W_out, b_out, tmpdir=None, trace=True, **_):
    """Like kernel() but returns (out, exec_time_ns); used by test.py."""
    nc = _build_program()
    in_maps = _make_in_maps(x, W_qkv, b_qkv, W_out)
    res = bass_utils.run_bass_kernel_spmd(
        nc, in_maps, core_ids=list(range(NCORES)), trace=trace, tmpdir=tmpdir
    )
    bias_extra = np.asarray(b_qkv, np.float32)[2 * C :] @ np.asarray(W_out, np.float32)
    return _gather(res.results, b_out, bias_extra), res.exec_time_ns
